# revision 78
# baseline (speedup 1.0000x reference)
"""Trainium2 Bass kernel for nn_Conv1dAttention.

Math (per sample):
  q,k,v,pe = lrelu(bn(conv1d(x, W_p)))           # [C=128, L=2048], Cin=64, K=3
  S = q^T k                                      # [L, L]
  P = softmax_rows(S)                            # softmax over last axis
  out = v @ P + pe                               # [C, L]

Sharding: data-parallel over batch B=16 across 8 NeuronCores (2 samples/core).
Same NEFF on all cores, per-core input shards, no collectives.

Design notes (v3 — dense exp stream):
  - ScalarE does NOTHING but the 64 exp ACTIVATEs (the ~73.4us floor).  All
    conv drains run on DVE; softmax row-sums Z come free via the ACTIVATE
    accum_out port, killing the old per-block DVE z pass and shortening the
    exp->vts->out latency chain to ~0.4us.
  - PSUM: psA ring 2x[128,1024] (S blocks + conv fillers) + psO [128,2048]
    out accumulator = exactly 8 banks.  Fillers are [128,512]-granular units
    inserted in PAIRS between blocks so the S allocations keep their slot
    parity (S(b+1,h0) always lands on the slot exp(b,h0) just freed).
  - A filler's PSUM slot is released by the TS bias pass alone (the STT max
    pass reads the SBUF scratch), so slot turnaround is ~1us.
  - Prelude: xs0 loaded in 2 halves (1-col overlap so each half is
    self-contained for the conv taps), first exp needs only q0 quarter0 +
    k0 cols 0:1024 -> first ACTIVATE at ~12us instead of ~27us.
  - v^T bias via ONE ones-row matmul per 512-col group (bv pre-tiled 4x in
    the weight pack) instead of one per 128-col block.
  - Weights arrive in 2 packed DMAs: w12ext carries the k0/k1-tap weights
    for all 4 convs plus the fp32 conv biases bitcast into bf16 columns;
    w3ext carries the k2-tap weights plus the tiled v-bias row.
  - out chase lags 1 block; the final block's out-matmuls interleave with
    the finish add+store per 512-col chunk.
"""

import sys

if "/opt/trn_rl_repo" not in sys.path:
    sys.path.insert(0, "/opt/trn_rl_repo")

from contextlib import ExitStack

import ml_dtypes
import numpy as np

import concourse.bass as bass
import concourse.tile as tile
from concourse import bacc, mybir
from concourse import dve_ops as _dvo
from concourse.bass_utils import run_bass_kernel_spmd
from concourse.dve_spec import C0 as _C0
from concourse.dve_spec import C1 as _C1
from concourse.dve_spec import Spec as _Spec
from concourse.dve_spec import Src0 as _Src0
from concourse.dve_spec import _has_src1 as _dve_has_src1
from concourse.dve_spec import lower as _dve_lower
from concourse.dve_spec import maxx as _dve_maxx
from concourse.dve_uop import DveOpSpec as _DveOpSpec


def _register_lrelu_op():
    """Fused out = lrelu(in + s0) custom DVE op: one 1x pass straight from
    PSUM replaces the TS bias-add + STT max pair.  Registered through the
    documented dve_ops extension points; sha pins computed at import."""
    for op in _dvo.OPS:
        if op.name == "ANT_LRELU_B":
            return op
    yb = _Src0 + _C0
    spec = _Spec(
        body=_dve_maxx(yb, yb * _C1),
        reference=lambda in0, in1, s0, s1, imm2: np.maximum(
            in0.astype(np.float32) + s0, (in0.astype(np.float32) + s0) * s1
        ),
    )
    name = "ANT_LRELU_B"
    row = _dvo._CUSTOM_DVE_ROW_BASE + len(_dvo.OPS)
    _dvo._SUB_OPCODE_FOR_NAME[name] = row
    shas = {}
    for ver in ("v3", "v4"):
        uops = _dve_lower(spec, ver=ver)
        shas[ver] = _DveOpSpec(
            name=name, opcode=row, uops=uops, rd1_en=_dve_has_src1(spec)
        ).sha(ver)
    op = _dvo.DveOp(name, spec, subdim=False, uops_sha=shas)
    _dvo.OPS.append(op)
    _dvo.CUSTOM_DVE_SPECS[name] = spec
    return op


_LRELU_B = _register_lrelu_op()

B, CIN, COUT, KW, L = 16, 64, 128, 3, 2048
NCORES = 8
BP = B // NCORES  # samples per core
EPS = 1e-5
SLOPE = 0.3
F32 = mybir.dt.float32
BF16 = mybir.dt.bfloat16
NB = L // 128  # 16 attention blocks
HALF = 1024

_CACHE = {}

PCOL = {"q": 0, "k": 1, "v": 2, "p": 3}


def _body(ctx, tc, x, w12e, w3e, out):
    nc = tc.nc
    amax = mybir.AluOpType.max
    mult = mybir.AluOpType.mult
    aadd = mybir.AluOpType.add
    Exp = mybir.ActivationFunctionType.Exp

    wpool = ctx.enter_context(tc.tile_pool(name="wpool", bufs=1))
    apool = ctx.enter_context(tc.tile_pool(name="apool", bufs=2))
    ppool = ctx.enter_context(tc.tile_pool(name="ppool", bufs=4))
    vpool = ctx.enter_context(tc.tile_pool(name="vpool", bufs=3))
    zpool = ctx.enter_context(tc.tile_pool(name="zpool", bufs=3))
    lpool = ctx.enter_context(tc.tile_pool(name="lpool", bufs=3))
    opool = ctx.enter_context(tc.tile_pool(name="opool", bufs=4))
    psA = ctx.enter_context(tc.tile_pool(name="psA", bufs=2, space="PSUM"))
    psO = ctx.enter_context(tc.tile_pool(name="psO", bufs=1, space="PSUM"))

    # --- persistent tiles
    # w12ext layout: [:, 0:512] k0/k1-tap weights (q,k,v,p); [:, 512:520] the
    # fp32 conv biases bitcast to bf16 pairs; [0, 520:1032] v-bias tiled 4x.
    w12_t = wpool.tile([128, 4 * COUT + 8 + 512 + 128], BF16, tag="w12", name="w12")
    w3_t = wpool.tile([CIN, 4 * COUT], BF16, tag="w3", name="w3")
    ones1 = wpool.tile([1, COUT], BF16, tag="ones1", name="ones1")
    wseed = wpool.tile([128, 512], BF16, tag="wseed", name="wseed")
    xs0 = wpool.tile([128, L + 1], BF16, tag="xs0", name="xs0")
    xs1 = wpool.tile([128, L + 1], BF16, tag="xs1", name="xs1")
    nc.gpsimd.memset(ones1[:, :], 1.0)
    nc.gpsimd.memset(wseed[:, :], 0.001)

    # conv biases: fp32 values bitcast into 8 bf16 cols of w12ext
    bc_ap = w12_t[:, 4 * COUT : 4 * COUT + 8].bitcast(F32)  # [128, 4] f32
    bv4row = w12_t[0:1, 4 * COUT + 8 : 4 * COUT + 8 + 512]  # v-bias tiled 4x
    ident = w12_t[:, 4 * COUT + 8 + 512 : 4 * COUT + 8 + 512 + 128]  # I_128

    # --- DMA issue order: xs0 half0 first (gates the first exp), weights on
    # the scalar queue in parallel, then xs0 half1, then xs1.  x arrives as a
    # host-prebuilt [128, L+1] image (rows 0-63 direct, rows 64-127 shifted,
    # zero edge columns baked in), so each half is ONE dma_start.
    nc.sync.dma_start(out=xs0[:, 0 : HALF + 1], in_=x[0, :, 0 : HALF + 1])
    nc.scalar.dma_start(out=w12_t[:, :], in_=w12e[:, :])
    nc.scalar.dma_start(out=w3_t[:, :], in_=w3e[:, :])
    nc.sync.dma_start(out=xs0[:, HALF + 1 : L + 1], in_=x[0, :, HALF + 1 : L + 1])
    nc.sync.dma_start(out=xs1[:, :], in_=x[1, :, :])

    def w12s(p):
        return w12_t[:, PCOL[p] * COUT : (PCOL[p] + 1) * COUT]

    def w3s(p):
        return w3_t[:, PCOL[p] * COUT : (PCOL[p] + 1) * COUT]

    def psa_tile(w, name):
        return psA.tile([128, w], F32, tag="ps", name=name)

    Prelu = mybir.ActivationFunctionType.Prelu

    def drain(dst_ap, ps_ap, p, eng="v"):
        # lrelu(y + b).  eng="s": single ScalarE Prelu (prelude only, before
        # the exp stream owns ScalarE).  eng="v": single fused custom DVE op
        # straight from PSUM (releases the psum slot in one 1x pass).
        if eng == "s":
            bias = bc_ap[:, PCOL[p] : PCOL[p] + 1] if p is not None else 0.0
            nc.scalar.activation(
                dst_ap, ps_ap, Prelu, bias=bias, scale=1.0, alpha=SLOPE
            )
            return
        bias = bc_ap[:, PCOL[p] : PCOL[p] + 1] if p is not None else 0.0
        nc.vector._custom_dve(
            _LRELU_B, out=dst_ap, in0=ps_ap, s0=bias, s1=SLOPE
        )

    def conv_q(xs, p, dst, q, eng="v"):
        # one [128,512] quarter of a [c,l]-layout conv: 1 psA alloc, 2 MMs
        cps = psa_tile(512, "cps")
        c0 = q * 512
        nc.tensor.matmul(cps[:, :], w12s(p), xs[:, c0 : c0 + 512], start=True, stop=False)
        nc.tensor.matmul(
            cps[:, :], w3s(p), xs[0:CIN, c0 + 1 : c0 + 513], start=False, stop=True
        )
        drain(dst[:, c0 : c0 + 512], cps[:, :], p, eng)

    def conv_h(xs, p, dst, h, eng="v"):
        # one [128,1024] half: 1 psA alloc, 4 MMs, one [1024] drain
        cps = psa_tile(HALF, "cph")
        for qq in range(2):
            c0 = h * HALF + qq * 512
            pc = slice(qq * 512, qq * 512 + 512)
            nc.tensor.matmul(
                cps[:, pc], w12s(p), xs[:, c0 : c0 + 512], start=True, stop=False
            )
            nc.tensor.matmul(
                cps[:, pc], w3s(p), xs[0:CIN, c0 + 1 : c0 + 513], start=False, stop=True
            )
        drain(dst[:, h * HALF : (h + 1) * HALF], cps[:, :], p, eng)

    def vt_group(xs, vt, gh, eng="v"):
        # 4 l-blocks of V in transposed [l,c] layout + ONE bias matmul.
        vps = psa_tile(512, "vps")
        for i in range(4):
            blk = gh * 4 + i
            c = blk * 128
            pc = slice(i * 128, i * 128 + 128)
            # start=True only on i==0: start clears has_written for the whole
            # PSUM bank, which would make the single trailing bias matmul
            # overwrite (not accumulate onto) the earlier sub-blocks.
            nc.tensor.matmul(
                vps[:, pc], xs[:, c : c + 128], w12s("v"), start=(i == 0), stop=False
            )
            nc.tensor.matmul(
                vps[:, pc], xs[0:CIN, c + 1 : c + 129], w3s("v"), start=False, stop=False
            )
        nc.tensor.matmul(vps[:, :], ones1[0:1, :], bv4row, start=False, stop=True)
        drain(vt[:, gh * 512 : (gh + 1) * 512], vps[:, :], None, eng)

    def make_tiles(s):
        q_t = apool.tile([128, L], BF16, tag="actq", name=f"q{s}")
        k_t = apool.tile([128, L], BF16, tag="actk", name=f"k{s}")
        pe_t = apool.tile([128, L], BF16, tag="actp", name=f"pe{s}")
        vt = apool.tile([128, L], BF16, tag="vt", name=f"vt{s}")
        return q_t, k_t, pe_t, vt

    def s_half(tiles, pblk, blk, h, acc=False):
        # S matmuls + exp (optionally with accumulated row-sum) for one
        # [128,1024] half.
        q_t, k_t = tiles[0], tiles[1]
        sps = psa_tile(HALF, "sps")
        for n in range(2):
            c0 = h * HALF + n * 512
            nc.tensor.matmul(
                sps[:, n * 512 : n * 512 + 512],
                q_t[:, blk * 128 : blk * 128 + 128],
                k_t[:, c0 : c0 + 512],
                start=True,
                stop=True,
            )
        outx = pblk[:, h * HALF : (h + 1) * HALF]
        if acc:
            zh = zpool.tile([128, 1], F32, tag=f"z{h}", name=f"z{h}")
            nc.scalar.activation(outx, sps[:, :], Exp, accum_out=zh[:, :])
            return zh
        nc.scalar.activation(outx, sps[:, :], Exp)
        return None

    def zfinish(tiles, blk, pblk, zpair):
        # softmax row-sum: either the two ACT-accumulator halves added (z came
        # free with the exps, used on blocks feeding a fillered successor so
        # DVE has room for the filler drains there), or one DVE STT add over
        # both P halves (two read ports -> 2048 elems in ~1024 cycles).
        vt = tiles[3]
        zs = zpool.tile([128, 1], F32, tag="zs", name="zs")
        if zpair[0] is not None:
            nc.vector.tensor_tensor(zs[:, :], zpair[0][:, :], zpair[1][:, :], aadd)
        else:
            zscr = lpool.tile([128, HALF], BF16, tag="zscr", name="zscr")
            nc.vector.scalar_tensor_tensor(
                zscr[:, :],
                pblk[:, 0:HALF],
                1.0,
                pblk[:, HALF:L],
                op0=mult,
                op1=aadd,
                accum_out=zs[:, :],
            )
        r = zpool.tile([128, 1], F32, tag="r", name="r")
        nc.vector.reciprocal(r[:, :], zs[:, :])
        vts = vpool.tile([128, 128], BF16, tag="vts", name="vts")
        nc.vector.tensor_scalar_mul(vts[:, :], vt[:, blk * 128 : blk * 128 + 128], r[:, :])
        return vts

    def out_mms(out_ps, pblk, vts, blk, finish=None, stop_last=True):
        # NOTE: PSUM dep tracking is tile-granular, so the finish TTs must
        # come AFTER all 4 matmuls (interleaving serializes MM(n+1) on TT(n))
        for n in range(4):
            nc.tensor.matmul(
                out_ps[:, n * 512 : n * 512 + 512],
                vts[:, :],
                pblk[:, n * 512 : n * 512 + 512],
                start=(blk == 0),
                stop=(blk == NB - 1 and stop_last),
            )
        if finish is not None:
            for n in range(4):
                finish(n)

    def finish_chunk(pe_t, out_ps, s, n):
        outs = opool.tile([128, 512], BF16, tag="outs", name=f"outc{n}")
        cols = slice(n * 512, (n + 1) * 512)
        nc.vector.tensor_tensor(outs[:, :], out_ps[:, cols], pe_t[:, cols], aadd)
        # sample 1's stores run after the exp stream ends: split them across
        # both HWDGE queues.  Sample 0's run mid-stream: keep them off the
        # scalar queue (it carries the exps).
        eng = nc.scalar if (s == 1 and n % 2 == 1) else nc.sync
        eng.dma_start(out=out[s, :, cols], in_=outs[:, :])

    # --- PE warm-up through the DMA wait.  HAM needs ~3.4us of sustained
    # activity to unthrottle; fine-grained N=128 matmuls keep the PE busy
    # right up to the xs0 arrival so the prelude convs run at 2.4 GHz.
    wps = psa_tile(512, "wps")
    for _ in range(16):
        nc.tensor.matmul(
            wps[:, 0:128], wseed[:, 0:128], wseed[:, 0:128], start=True, stop=True
        )

    tiles0 = make_tiles(0)
    q0, k0, pe0, vt0 = tiles0
    tiles1 = make_tiles(1)
    q1, k1, pe1, vt1 = tiles1

    # --- prelude: exactly what exp(b0) needs, nothing else.  Drains run as
    # single-pass ScalarE Prelus: ScalarE is idle until the first exp.
    conv_q(xs0, "q", q0, 0, eng="s")
    conv_q(xs0, "k", k0, 0, eng="s")
    conv_q(xs0, "k", k0, 1, eng="s")

    pblk0 = ppool.tile([128, L], BF16, tag="pblk", name="pblk0")
    zh0 = s_half(tiles0, pblk0, 0, 0)
    conv_q(xs0, "k", k0, 2, eng="s")
    conv_q(xs0, "k", k0, 3, eng="s")
    zh1 = s_half(tiles0, pblk0, 0, 1)
    # vt0 g0 must precede zfinish(b0); still in the ScalarE-drained prelude.
    vt_group(xs0, vt0, 0, eng="s")
    conv_q(xs0, "q", q0, 1, eng="s")

    # filler schedule: ONE [512/1024]-granular psA unit per block, drained
    # by a ScalarE Prelu riding the exp stream — the slot release happens on
    # the same engine as the exps, so the next block's S matmuls never wait
    # on a cross-engine drain.  Deadlines: vt0 g_i before block 4i+1's
    # zfinish; q0 quarter i before block 4i; k1/q1/vt1g0 before phase C;
    # pe0 before C-block 2's finish; pe1 before the C tail.
    fillB = {
        1: [lambda: vt_group(xs0, vt0, 1, eng="s")],
        2: [lambda: conv_q(xs0, "q", q0, 2, eng="s")],
        3: [lambda: vt_group(xs0, vt0, 2, eng="s")],
        4: [lambda: conv_q(xs0, "q", q0, 3, eng="s")],
        5: [lambda: vt_group(xs0, vt0, 3, eng="s")],
        6: [lambda: conv_h(xs1, "k", k1, 0, eng="s")],
        8: [lambda: conv_h(xs1, "k", k1, 1, eng="s")],
        10: [lambda: conv_h(xs1, "q", q1, 0, eng="s")],
        12: [lambda: conv_h(xs1, "q", q1, 1, eng="s")],
        14: [lambda: conv_h(xs0, "p", pe0, 0, eng="s")],
    }
    fillC = {
        1: [lambda: conv_q(xs1, "p", pe1, 0, eng="s")],
        2: [lambda: conv_q(xs1, "p", pe1, 1, eng="s")],
        3: [lambda: vt_group(xs1, vt1, 1, eng="s")],
        4: [lambda: vt_group(xs1, vt1, 2, eng="s")],
        6: [lambda: vt_group(xs1, vt1, 3, eng="s")],
        8: [lambda: conv_h(xs1, "p", pe1, 1, eng="s")],
    }

    def attention_phase(tiles, out_ps, fillers, first_pblk, first_z, carry, tail_acc=False):
        """Blocks 1..15 of one sample; block 0's S/exp already emitted.
        carry = cross-phase PE work (previous sample's trailing outs),
        drained one item per block.  Returns last block's (pend, pblk, vts).
        Blocks preceding a fillered block use the ScalarE accumulator for z
        so DVE is free for the filler drains."""
        pblk_prev = first_pblk
        z_prev = first_z
        pend = []  # (pblk, vts, blk) awaiting out_mms, lag 2
        for blk in range(1, NB):
            # last two blocks use the ScalarE accumulator for z so the tail's
            # zfinish -> out -> store chain starts right after the final exp
            acc = tail_acc and blk >= NB - 2
            pblk = ppool.tile([128, L], BF16, tag="pblk", name=f"pblk{blk}")
            # fillers at the BLOCK TOP: their psA slot was freed by the
            # previous block's first exp, so the conv matmuls get a full
            # activation of lead time and the in-stream Prelu never idles
            for f in fillers.get(blk, []):
                f()
            za = s_half(tiles, pblk, blk, 0, acc)
            zb = s_half(tiles, pblk, blk, 1, acc)
            vts_prev = zfinish(tiles, blk - 1, pblk_prev, z_prev)
            pend.append((pblk_prev, vts_prev, blk - 1))
            if carry:
                carry.pop(0)()
            if len(pend) > 1:
                p, v, bb = pend.pop(0)
                out_mms(out_ps, p, v, bb)
            pblk_prev = pblk
            z_prev = (za, zb)
        # last block: zfinish; pending outs emitted by caller
        vts_last = zfinish(tiles, NB - 1, pblk_prev, z_prev)
        return pend, pblk_prev, vts_last

    out_ps0 = psO.tile([128, L], F32, tag="ops", name="out_ps0")
    pend0, pblkL0, vtsL0 = attention_phase(
        tiles0, out_ps0, fillB, pblk0, (zh0, zh1), []
    )

    # --- phase C: sample 1's S/exp starts immediately; sample 0's trailing
    # outs + finish ride along as carry work.
    pblk0c = ppool.tile([128, L], BF16, tag="pblk", name="pblk0c")
    zh0c = s_half(tiles1, pblk0c, 0, 0)
    zh1c = s_half(tiles1, pblk0c, 0, 1)
    # transition units: last pe0 half + vt1 g0 (needed by zfinish(C-b0)
    # in C block 1)
    conv_h(xs0, "p", pe0, 1, eng="s")
    vt_group(xs1, vt1, 0, eng="s")

    carry = []
    for p, v, bb in pend0:
        carry.append(lambda p=p, v=v, bb=bb: out_mms(out_ps0, p, v, bb))
    carry.append(
        lambda: out_mms(
            out_ps0, pblkL0, vtsL0, NB - 1,
            finish=lambda n: finish_chunk(pe0, out_ps0, 0, n),
        )
    )

    out_ps1 = psO.tile([128, L], F32, tag="ops", name="out_ps1")
    pend1, pblkL1, vtsL1 = attention_phase(
        tiles1, out_ps1, fillC, pblk0c, (zh0c, zh1c), carry, tail_acc=True
    )
    for p, v, bb in pend1:
        out_mms(out_ps1, p, v, bb)
    # tail: keep the accumulation open, fold the +pe via identity matmuls on
    # the now-idle PE, then split the psum->bf16 copies across the idle
    # ScalarE and the DVE, stores across both HWDGE queues.
    out_mms(out_ps1, pblkL1, vtsL1, NB - 1, stop_last=False)
    Copy = mybir.ActivationFunctionType.Copy
    for n in range(4):
        cols = slice(n * 512, (n + 1) * 512)
        nc.tensor.matmul(out_ps1[:, cols], ident, pe1[:, cols], start=False, stop=True)
    for n in (0, 2, 1, 3):
        cols = slice(n * 512, (n + 1) * 512)
        outs = opool.tile([128, 512], BF16, tag="outs", name=f"outf{n}")
        if n % 2 == 0:
            nc.scalar.activation(outs[:, :], out_ps1[:, cols], Copy)
            nc.scalar.dma_start(out=out[1, :, cols], in_=outs[:, :])
        else:
            nc.vector.tensor_scalar_mul(outs[:, :], out_ps1[:, cols], 1.0)
            nc.sync.dma_start(out=out[1, :, cols], in_=outs[:, :])


def build():
    nc = bacc.Bacc("TRN2", target_bir_lowering=False, debug=False)
    # x arrives as the prebuilt xs SBUF image: rows 0-63 = x (tap k=1/k=2),
    # rows 64-127 = x shifted right one col (tap k=0), zero edge cols baked.
    x_d = nc.dram_tensor("x", [BP, 128, L + 1], BF16, kind="ExternalInput")
    w12_d = nc.dram_tensor(
        "w12e", [128, 4 * COUT + 8 + 512 + 128], BF16, kind="ExternalInput"
    )
    w3_d = nc.dram_tensor("w3e", [CIN, 4 * COUT], BF16, kind="ExternalInput")
    out_d = nc.dram_tensor("out", [BP, COUT, L], BF16, kind="ExternalOutput")

    with tile.TileContext(nc) as tc, ExitStack() as ctx:
        _body(ctx, tc, x_d.ap(), w12_d.ap(), w3_d.ap(), out_d.ap())
    nc.compile()
    return nc


def _fold_weights(w, b, gamma, beta, mean, var):
    """Fold BN affine (fixed mean/var) into conv weights; split by tap."""
    w = np.asarray(w, np.float64)
    scale = np.asarray(gamma, np.float64) / np.sqrt(np.asarray(var, np.float64) + EPS)
    shift = np.asarray(beta, np.float64) - np.asarray(mean, np.float64) * scale
    wf = w * scale[:, None, None]  # [COUT, CIN, K]
    bf = np.asarray(b, np.float64) * scale + shift
    w12 = np.empty((128, COUT), np.float32)
    w12[0:CIN] = wf[:, :, 1].T
    w12[CIN:128] = wf[:, :, 0].T
    w3 = np.ascontiguousarray(wf[:, :, 2].T.astype(np.float32))  # [CIN, COUT]
    return w12, w3, bf.astype(np.float32)


def _get_nc():
    if "nc" not in _CACHE:
        _CACHE["nc"] = build()
    return _CACHE["nc"]


def make_in_maps(inputs):
    bf = ml_dtypes.bfloat16
    xf = np.asarray(inputs["x"], np.float32).astype(bf)
    x = np.zeros((B, 128, L + 1), dtype=bf)
    x[:, 0:CIN, 0:L] = xf
    x[:, CIN:128, 1 : L + 1] = xf
    folded = {}
    for p in "qkvp":
        key = p if p != "p" else "pe"
        folded[p] = _fold_weights(
            inputs[f"{key}_w"],
            inputs[f"{key}_b"],
            inputs[f"{key}_gamma"],
            inputs[f"{key}_beta"],
            inputs[f"{key}_mean"],
            inputs[f"{key}_var"],
        )
    w12pack = np.concatenate([folded[p][0] for p in "qkvp"], axis=1).astype(bf)
    bcols = np.stack([folded[p][2] for p in "qkvp"], axis=1).astype(np.float32)
    w12e = np.zeros((128, 4 * COUT + 8 + 512 + 128), dtype=bf)
    w12e[:, 4 * COUT + 8 + 512 :] = np.eye(128, dtype=np.float32)
    w12e[:, 0 : 4 * COUT] = w12pack
    w12e[:, 4 * COUT : 4 * COUT + 8] = np.ascontiguousarray(bcols).view(bf)
    w12e[0, 4 * COUT + 8 : 4 * COUT + 8 + 512] = np.tile(folded["v"][2].astype(bf), 4)
    w3e = np.ascontiguousarray(
        np.concatenate([folded[p][1] for p in "qkvp"], axis=1).astype(bf)
    )
    in_maps = []
    for i in range(NCORES):
        m = {
            "x": np.ascontiguousarray(x[i * BP : (i + 1) * BP]),
            "w12e": w12e,
            "w3e": w3e,
        }
        in_maps.append(m)
    return in_maps


def kernel(**inputs):
    nc = _get_nc()
    in_maps = make_in_maps(inputs)
    res = run_bass_kernel_spmd(nc, in_maps, core_ids=list(range(NCORES)))
    out = np.concatenate([res.results[i]["out"] for i in range(NCORES)], axis=0)
    return out.astype(np.float32)


if __name__ == "__main__":
    rng = np.random.default_rng(0)
    ins = {"x": rng.standard_normal((B, CIN, L), dtype=np.float32)}
    for p in ("q", "k", "v", "pe"):
        ins[f"{p}_w"] = (rng.standard_normal((COUT, CIN, KW)) * 0.05).astype(np.float32)
        ins[f"{p}_b"] = (rng.standard_normal(COUT) * 0.05).astype(np.float32)
        ins[f"{p}_gamma"] = rng.uniform(0.5, 1.5, COUT).astype(np.float32)
        ins[f"{p}_beta"] = (rng.standard_normal(COUT) * 0.05).astype(np.float32)
        ins[f"{p}_mean"] = (rng.standard_normal(COUT) * 0.05).astype(np.float32)
        ins[f"{p}_var"] = rng.uniform(0.5, 1.5, COUT).astype(np.float32)
    got = kernel(**ins)
    print("kernel output:", got.shape, got.dtype, np.abs(got).mean())


# revision 79
# speedup vs baseline: 1.0083x; 1.0083x over previous
"""Trainium2 Bass kernel for nn_Conv1dAttention.

Math (per sample):
  q,k,v,pe = lrelu(bn(conv1d(x, W_p)))           # [C=128, L=2048], Cin=64, K=3
  S = q^T k                                      # [L, L]
  P = softmax_rows(S)                            # softmax over last axis
  out = v @ P + pe                               # [C, L]

Sharding: data-parallel over batch B=16 across 8 NeuronCores (2 samples/core).
Same NEFF on all cores, per-core input shards, no collectives.

Design notes (v3 — dense exp stream):
  - ScalarE does NOTHING but the 64 exp ACTIVATEs (the ~73.4us floor).  All
    conv drains run on DVE; softmax row-sums Z come free via the ACTIVATE
    accum_out port, killing the old per-block DVE z pass and shortening the
    exp->vts->out latency chain to ~0.4us.
  - PSUM: psA ring 2x[128,1024] (S blocks + conv fillers) + psO [128,2048]
    out accumulator = exactly 8 banks.  Fillers are [128,512]-granular units
    inserted in PAIRS between blocks so the S allocations keep their slot
    parity (S(b+1,h0) always lands on the slot exp(b,h0) just freed).
  - A filler's PSUM slot is released by the TS bias pass alone (the STT max
    pass reads the SBUF scratch), so slot turnaround is ~1us.
  - Prelude: xs0 loaded in 2 halves (1-col overlap so each half is
    self-contained for the conv taps), first exp needs only q0 quarter0 +
    k0 cols 0:1024 -> first ACTIVATE at ~12us instead of ~27us.
  - v^T bias via ONE ones-row matmul per 512-col group (bv pre-tiled 4x in
    the weight pack) instead of one per 128-col block.
  - Weights arrive in 2 packed DMAs: w12ext carries the k0/k1-tap weights
    for all 4 convs plus the fp32 conv biases bitcast into bf16 columns;
    w3ext carries the k2-tap weights plus the tiled v-bias row.
  - out chase lags 1 block; the final block's out-matmuls interleave with
    the finish add+store per 512-col chunk.
"""

import sys

if "/opt/trn_rl_repo" not in sys.path:
    sys.path.insert(0, "/opt/trn_rl_repo")

from contextlib import ExitStack

import ml_dtypes
import numpy as np

import concourse.bass as bass
import concourse.tile as tile
from concourse import bacc, mybir
from concourse import dve_ops as _dvo
from concourse.bass_utils import run_bass_kernel_spmd
from concourse.dve_spec import C0 as _C0
from concourse.dve_spec import C1 as _C1
from concourse.dve_spec import Spec as _Spec
from concourse.dve_spec import Src0 as _Src0
from concourse.dve_spec import _has_src1 as _dve_has_src1
from concourse.dve_spec import lower as _dve_lower
from concourse.dve_spec import maxx as _dve_maxx
from concourse.dve_uop import DveOpSpec as _DveOpSpec


def _register_lrelu_op():
    """Fused out = lrelu(in + s0) custom DVE op: one 1x pass straight from
    PSUM replaces the TS bias-add + STT max pair.  Registered through the
    documented dve_ops extension points; sha pins computed at import."""
    for op in _dvo.OPS:
        if op.name == "ANT_LRELU_B":
            return op
    yb = _Src0 + _C0
    spec = _Spec(
        body=_dve_maxx(yb, yb * _C1),
        reference=lambda in0, in1, s0, s1, imm2: np.maximum(
            in0.astype(np.float32) + s0, (in0.astype(np.float32) + s0) * s1
        ),
    )
    name = "ANT_LRELU_B"
    row = _dvo._CUSTOM_DVE_ROW_BASE + len(_dvo.OPS)
    _dvo._SUB_OPCODE_FOR_NAME[name] = row
    shas = {}
    for ver in ("v3", "v4"):
        uops = _dve_lower(spec, ver=ver)
        shas[ver] = _DveOpSpec(
            name=name, opcode=row, uops=uops, rd1_en=_dve_has_src1(spec)
        ).sha(ver)
    op = _dvo.DveOp(name, spec, subdim=False, uops_sha=shas)
    _dvo.OPS.append(op)
    _dvo.CUSTOM_DVE_SPECS[name] = spec
    return op


_LRELU_B = _register_lrelu_op()

B, CIN, COUT, KW, L = 16, 64, 128, 3, 2048
NCORES = 8
BP = B // NCORES  # samples per core
EPS = 1e-5
SLOPE = 0.3
F32 = mybir.dt.float32
BF16 = mybir.dt.bfloat16
NB = L // 128  # 16 attention blocks
HALF = 1024

_CACHE = {}

PCOL = {"q": 0, "k": 1, "v": 2, "p": 3}


def _body(ctx, tc, x, w12e, w3e, out):
    nc = tc.nc
    amax = mybir.AluOpType.max
    mult = mybir.AluOpType.mult
    aadd = mybir.AluOpType.add
    Exp = mybir.ActivationFunctionType.Exp

    wpool = ctx.enter_context(tc.tile_pool(name="wpool", bufs=1))
    apool = ctx.enter_context(tc.tile_pool(name="apool", bufs=2))
    ppool = ctx.enter_context(tc.tile_pool(name="ppool", bufs=4))
    vpool = ctx.enter_context(tc.tile_pool(name="vpool", bufs=3))
    zpool = ctx.enter_context(tc.tile_pool(name="zpool", bufs=3))
    lpool = ctx.enter_context(tc.tile_pool(name="lpool", bufs=3))
    opool = ctx.enter_context(tc.tile_pool(name="opool", bufs=4))
    psA = ctx.enter_context(tc.tile_pool(name="psA", bufs=2, space="PSUM"))
    psO = ctx.enter_context(tc.tile_pool(name="psO", bufs=1, space="PSUM"))

    # --- persistent tiles
    # w12ext layout: [:, 0:512] k0/k1-tap weights (q,k,v,p); [:, 512:520] the
    # fp32 conv biases bitcast to bf16 pairs; [0, 520:1032] v-bias tiled 4x.
    w12_t = wpool.tile([128, 4 * COUT + 8 + 512 + 128], BF16, tag="w12", name="w12")
    w3_t = wpool.tile([CIN, 4 * COUT], BF16, tag="w3", name="w3")
    ones1 = wpool.tile([1, COUT], BF16, tag="ones1", name="ones1")
    wseed = wpool.tile([128, 512], BF16, tag="wseed", name="wseed")
    xs0 = wpool.tile([128, L + 1], BF16, tag="xs0", name="xs0")
    xs1 = wpool.tile([128, L + 1], BF16, tag="xs1", name="xs1")
    nc.gpsimd.memset(ones1[:, :], 1.0)
    nc.gpsimd.memset(wseed[:, :], 0.001)

    # conv biases: fp32 values bitcast into 8 bf16 cols of w12ext
    bc_ap = w12_t[:, 4 * COUT : 4 * COUT + 8].bitcast(F32)  # [128, 4] f32
    bv4row = w12_t[0:1, 4 * COUT + 8 : 4 * COUT + 8 + 512]  # v-bias tiled 4x
    ident = w12_t[:, 4 * COUT + 8 + 512 : 4 * COUT + 8 + 512 + 128]  # I_128

    # --- DMA issue order: xs0 half0 first (gates the first exp), weights on
    # the scalar queue in parallel, then xs0 half1, then xs1.  x arrives as a
    # host-prebuilt [128, L+1] image (rows 0-63 direct, rows 64-127 shifted,
    # zero edge columns baked in), so each half is ONE dma_start.
    nc.sync.dma_start(out=xs0[:, 0 : HALF + 1], in_=x[0, :, 0 : HALF + 1])
    nc.scalar.dma_start(out=w12_t[:, :], in_=w12e[:, :])
    nc.scalar.dma_start(out=w3_t[:, :], in_=w3e[:, :])
    nc.sync.dma_start(out=xs0[:, HALF + 1 : L + 1], in_=x[0, :, HALF + 1 : L + 1])
    nc.sync.dma_start(out=xs1[:, :], in_=x[1, :, :])

    def w12s(p):
        return w12_t[:, PCOL[p] * COUT : (PCOL[p] + 1) * COUT]

    def w3s(p):
        return w3_t[:, PCOL[p] * COUT : (PCOL[p] + 1) * COUT]

    def psa_tile(w, name):
        return psA.tile([128, w], F32, tag="ps", name=name)

    Prelu = mybir.ActivationFunctionType.Prelu

    def drain(dst_ap, ps_ap, p, eng="v"):
        # lrelu(y + b).  eng="s": single ScalarE Prelu (prelude only, before
        # the exp stream owns ScalarE).  eng="v": single fused custom DVE op
        # straight from PSUM (releases the psum slot in one 1x pass).
        if eng == "s":
            bias = bc_ap[:, PCOL[p] : PCOL[p] + 1] if p is not None else 0.0
            nc.scalar.activation(
                dst_ap, ps_ap, Prelu, bias=bias, scale=1.0, alpha=SLOPE
            )
            return
        bias = bc_ap[:, PCOL[p] : PCOL[p] + 1] if p is not None else 0.0
        nc.vector._custom_dve(
            _LRELU_B, out=dst_ap, in0=ps_ap, s0=bias, s1=SLOPE
        )

    def conv_q(xs, p, dst, q, eng="v"):
        # one [128,512] quarter of a [c,l]-layout conv: 1 psA alloc, 2 MMs
        cps = psa_tile(512, "cps")
        c0 = q * 512
        nc.tensor.matmul(cps[:, :], w12s(p), xs[:, c0 : c0 + 512], start=True, stop=False)
        nc.tensor.matmul(
            cps[:, :], w3s(p), xs[0:CIN, c0 + 1 : c0 + 513], start=False, stop=True
        )
        drain(dst[:, c0 : c0 + 512], cps[:, :], p, eng)

    def conv_h(xs, p, dst, h, eng="v"):
        # one [128,1024] half: 1 psA alloc, 4 MMs, one [1024] drain
        cps = psa_tile(HALF, "cph")
        for qq in range(2):
            c0 = h * HALF + qq * 512
            pc = slice(qq * 512, qq * 512 + 512)
            nc.tensor.matmul(
                cps[:, pc], w12s(p), xs[:, c0 : c0 + 512], start=True, stop=False
            )
            nc.tensor.matmul(
                cps[:, pc], w3s(p), xs[0:CIN, c0 + 1 : c0 + 513], start=False, stop=True
            )
        drain(dst[:, h * HALF : (h + 1) * HALF], cps[:, :], p, eng)

    def vt_group(xs, vt, gh, eng="v"):
        # 4 l-blocks of V in transposed [l,c] layout + ONE bias matmul.
        vps = psa_tile(512, "vps")
        for i in range(4):
            blk = gh * 4 + i
            c = blk * 128
            pc = slice(i * 128, i * 128 + 128)
            # start=True only on i==0: start clears has_written for the whole
            # PSUM bank, which would make the single trailing bias matmul
            # overwrite (not accumulate onto) the earlier sub-blocks.
            nc.tensor.matmul(
                vps[:, pc], xs[:, c : c + 128], w12s("v"), start=(i == 0), stop=False
            )
            nc.tensor.matmul(
                vps[:, pc], xs[0:CIN, c + 1 : c + 129], w3s("v"), start=False, stop=False
            )
        nc.tensor.matmul(vps[:, :], ones1[0:1, :], bv4row, start=False, stop=True)
        drain(vt[:, gh * 512 : (gh + 1) * 512], vps[:, :], None, eng)

    def make_tiles(s):
        q_t = apool.tile([128, L], BF16, tag="actq", name=f"q{s}")
        k_t = apool.tile([128, L], BF16, tag="actk", name=f"k{s}")
        pe_t = apool.tile([128, L], BF16, tag="actp", name=f"pe{s}")
        vt = apool.tile([128, L], BF16, tag="vt", name=f"vt{s}")
        return q_t, k_t, pe_t, vt

    def s_half(tiles, pblk, blk, h, acc=False):
        # S matmuls + exp (optionally with accumulated row-sum) for one
        # [128,1024] half.
        q_t, k_t = tiles[0], tiles[1]
        sps = psa_tile(HALF, "sps")
        for n in range(2):
            c0 = h * HALF + n * 512
            nc.tensor.matmul(
                sps[:, n * 512 : n * 512 + 512],
                q_t[:, blk * 128 : blk * 128 + 128],
                k_t[:, c0 : c0 + 512],
                start=True,
                stop=True,
            )
        outx = pblk[:, h * HALF : (h + 1) * HALF]
        if acc:
            zh = zpool.tile([128, 1], F32, tag=f"z{h}", name=f"z{h}")
            nc.scalar.activation(outx, sps[:, :], Exp, accum_out=zh[:, :])
            return zh
        nc.scalar.activation(outx, sps[:, :], Exp)
        return None

    def zfinish(tiles, blk, pblk, zpair):
        # softmax row-sum: either the two ACT-accumulator halves added (z came
        # free with the exps, used on blocks feeding a fillered successor so
        # DVE has room for the filler drains there), or one DVE STT add over
        # both P halves (two read ports -> 2048 elems in ~1024 cycles).
        vt = tiles[3]
        zs = zpool.tile([128, 1], F32, tag="zs", name="zs")
        if zpair[0] is not None:
            nc.vector.tensor_tensor(zs[:, :], zpair[0][:, :], zpair[1][:, :], aadd)
        else:
            zscr = lpool.tile([128, HALF], BF16, tag="zscr", name="zscr")
            nc.vector.scalar_tensor_tensor(
                zscr[:, :],
                pblk[:, 0:HALF],
                1.0,
                pblk[:, HALF:L],
                op0=mult,
                op1=aadd,
                accum_out=zs[:, :],
            )
        r = zpool.tile([128, 1], F32, tag="r", name="r")
        nc.vector.reciprocal(r[:, :], zs[:, :])
        vts = vpool.tile([128, 128], BF16, tag="vts", name="vts")
        nc.vector.tensor_scalar_mul(vts[:, :], vt[:, blk * 128 : blk * 128 + 128], r[:, :])
        return vts

    def out_mms(out_ps, pblk, vts, blk, finish=None, stop_last=True):
        # NOTE: PSUM dep tracking is tile-granular, so the finish TTs must
        # come AFTER all 4 matmuls (interleaving serializes MM(n+1) on TT(n))
        for n in range(4):
            nc.tensor.matmul(
                out_ps[:, n * 512 : n * 512 + 512],
                vts[:, :],
                pblk[:, n * 512 : n * 512 + 512],
                start=(blk == 0),
                stop=(blk == NB - 1 and stop_last),
            )
        if finish is not None:
            for n in range(4):
                finish(n)

    def finish_chunk(pe_t, out_ps, s, n):
        outs = opool.tile([128, 512], BF16, tag="outs", name=f"outc{n}")
        cols = slice(n * 512, (n + 1) * 512)
        nc.vector.tensor_tensor(outs[:, :], out_ps[:, cols], pe_t[:, cols], aadd)
        # sample 1's stores run after the exp stream ends: split them across
        # both HWDGE queues.  Sample 0's run mid-stream: keep them off the
        # scalar queue (it carries the exps).
        eng = nc.scalar if (s == 1 and n % 2 == 1) else nc.sync
        eng.dma_start(out=out[s, :, cols], in_=outs[:, :])

    # --- PE warm-up through the DMA wait.  HAM needs ~3.4us of sustained
    # activity to unthrottle; fine-grained N=128 matmuls keep the PE busy
    # right up to the xs0 arrival so the prelude convs run at 2.4 GHz.
    wps = psa_tile(512, "wps")
    for _ in range(16):
        nc.tensor.matmul(
            wps[:, 0:128], wseed[:, 0:128], wseed[:, 0:128], start=True, stop=True
        )

    tiles0 = make_tiles(0)
    q0, k0, pe0, vt0 = tiles0
    tiles1 = make_tiles(1)
    q1, k1, pe1, vt1 = tiles1

    # --- prelude: exactly what exp(b0) needs, nothing else.  Drains run as
    # single-pass ScalarE Prelus: ScalarE is idle until the first exp.
    conv_q(xs0, "q", q0, 0, eng="s")
    conv_q(xs0, "k", k0, 0, eng="s")
    conv_q(xs0, "k", k0, 1, eng="s")

    pblk0 = ppool.tile([128, L], BF16, tag="pblk", name="pblk0")
    zh0 = s_half(tiles0, pblk0, 0, 0)
    conv_q(xs0, "k", k0, 2, eng="s")
    conv_q(xs0, "k", k0, 3, eng="s")
    zh1 = s_half(tiles0, pblk0, 0, 1)
    # vt0 g0 must precede zfinish(b0); still in the ScalarE-drained prelude.
    vt_group(xs0, vt0, 0, eng="s")
    conv_q(xs0, "q", q0, 1, eng="s")

    # filler schedule: ONE [512/1024]-granular psA unit per block, drained
    # by a ScalarE Prelu riding the exp stream — the slot release happens on
    # the same engine as the exps, so the next block's S matmuls never wait
    # on a cross-engine drain.  Deadlines: vt0 g_i before block 4i+1's
    # zfinish; q0 quarter i before block 4i; k1/q1/vt1g0 before phase C;
    # pe0 before C-block 2's finish; pe1 before the C tail.
    fillB = {
        1: [lambda: vt_group(xs0, vt0, 1, eng="s")],
        2: [lambda: conv_q(xs0, "q", q0, 2, eng="s")],
        3: [lambda: vt_group(xs0, vt0, 2, eng="s")],
        4: [lambda: conv_q(xs0, "q", q0, 3, eng="s")],
        5: [lambda: vt_group(xs0, vt0, 3, eng="s")],
        6: [lambda: conv_h(xs1, "k", k1, 0, eng="s")],
        8: [lambda: conv_h(xs1, "k", k1, 1, eng="s")],
        10: [lambda: conv_h(xs1, "q", q1, 0, eng="s")],
        12: [lambda: conv_h(xs1, "q", q1, 1, eng="s")],
        14: [lambda: conv_h(xs0, "p", pe0, 0, eng="s")],
    }
    fillC = {
        1: [lambda: conv_q(xs1, "p", pe1, 0, eng="s")],
        2: [lambda: conv_q(xs1, "p", pe1, 1, eng="s")],
        3: [lambda: vt_group(xs1, vt1, 1, eng="s")],
        4: [lambda: vt_group(xs1, vt1, 2, eng="s")],
        6: [lambda: vt_group(xs1, vt1, 3, eng="s")],
        8: [lambda: conv_h(xs1, "p", pe1, 1, eng="s")],
    }

    def attention_phase(tiles, out_ps, fillers, first_pblk, first_z, carry, tail_acc=False):
        """Blocks 1..15 of one sample; block 0's S/exp already emitted.
        carry = cross-phase PE work (previous sample's trailing outs),
        drained one item per block.  Returns last block's (pend, pblk, vts).
        Blocks preceding a fillered block use the ScalarE accumulator for z
        so DVE is free for the filler drains."""
        pblk_prev = first_pblk
        z_prev = first_z
        pend = []  # (pblk, vts, blk) awaiting out_mms, lag 2
        for blk in range(1, NB):
            # last two blocks use the ScalarE accumulator for z so the tail's
            # zfinish -> out -> store chain starts right after the final exp
            acc = tail_acc and blk >= NB - 2
            pblk = ppool.tile([128, L], BF16, tag="pblk", name=f"pblk{blk}")
            # fillers at the BLOCK TOP: their psA slot was freed by the
            # previous block's first exp, so the conv matmuls get a full
            # activation of lead time and the in-stream Prelu never idles
            for f in fillers.get(blk, []):
                f()
            za = s_half(tiles, pblk, blk, 0, acc)
            zb = s_half(tiles, pblk, blk, 1, acc)
            vts_prev = zfinish(tiles, blk - 1, pblk_prev, z_prev)
            pend.append((pblk_prev, vts_prev, blk - 1))
            if carry:
                carry.pop(0)()
            if len(pend) > 1:
                p, v, bb = pend.pop(0)
                out_mms(out_ps, p, v, bb)
            pblk_prev = pblk
            z_prev = (za, zb)
        # last block: zfinish; pending outs emitted by caller
        vts_last = zfinish(tiles, NB - 1, pblk_prev, z_prev)
        return pend, pblk_prev, vts_last

    out_ps0 = psO.tile([128, L], F32, tag="ops", name="out_ps0")
    pend0, pblkL0, vtsL0 = attention_phase(
        tiles0, out_ps0, fillB, pblk0, (zh0, zh1), []
    )

    # --- phase C: sample 1's S/exp starts immediately; sample 0's trailing
    # outs + finish ride along as carry work.
    pblk0c = ppool.tile([128, L], BF16, tag="pblk", name="pblk0c")
    zh0c = s_half(tiles1, pblk0c, 0, 0)
    zh1c = s_half(tiles1, pblk0c, 0, 1)
    # transition units: last pe0 half + vt1 g0 (needed by zfinish(C-b0)
    # in C block 1)
    conv_h(xs0, "p", pe0, 1, eng="s")
    vt_group(xs1, vt1, 0, eng="s")

    carry = []
    for p, v, bb in pend0:
        carry.append(lambda p=p, v=v, bb=bb: out_mms(out_ps0, p, v, bb))
    carry.append(
        lambda: out_mms(
            out_ps0, pblkL0, vtsL0, NB - 1,
            finish=lambda n: finish_chunk(pe0, out_ps0, 0, n),
        )
    )

    out_ps1 = psO.tile([128, L], F32, tag="ops", name="out_ps1")
    pend1, pblkL1, vtsL1 = attention_phase(
        tiles1, out_ps1, fillC, pblk0c, (zh0c, zh1c), carry, tail_acc=True
    )
    for p, v, bb in pend1:
        out_mms(out_ps1, p, v, bb)
    out_mms(
        out_ps1, pblkL1, vtsL1, NB - 1,
        finish=lambda n: finish_chunk(pe1, out_ps1, 1, n),
    )


def build():
    nc = bacc.Bacc("TRN2", target_bir_lowering=False, debug=False)
    # x arrives as the prebuilt xs SBUF image: rows 0-63 = x (tap k=1/k=2),
    # rows 64-127 = x shifted right one col (tap k=0), zero edge cols baked.
    x_d = nc.dram_tensor("x", [BP, 128, L + 1], BF16, kind="ExternalInput")
    w12_d = nc.dram_tensor(
        "w12e", [128, 4 * COUT + 8 + 512 + 128], BF16, kind="ExternalInput"
    )
    w3_d = nc.dram_tensor("w3e", [CIN, 4 * COUT], BF16, kind="ExternalInput")
    out_d = nc.dram_tensor("out", [BP, COUT, L], BF16, kind="ExternalOutput")

    with tile.TileContext(nc) as tc, ExitStack() as ctx:
        _body(ctx, tc, x_d.ap(), w12_d.ap(), w3_d.ap(), out_d.ap())
    nc.compile()
    return nc


def _fold_weights(w, b, gamma, beta, mean, var):
    """Fold BN affine (fixed mean/var) into conv weights; split by tap."""
    w = np.asarray(w, np.float64)
    scale = np.asarray(gamma, np.float64) / np.sqrt(np.asarray(var, np.float64) + EPS)
    shift = np.asarray(beta, np.float64) - np.asarray(mean, np.float64) * scale
    wf = w * scale[:, None, None]  # [COUT, CIN, K]
    bf = np.asarray(b, np.float64) * scale + shift
    w12 = np.empty((128, COUT), np.float32)
    w12[0:CIN] = wf[:, :, 1].T
    w12[CIN:128] = wf[:, :, 0].T
    w3 = np.ascontiguousarray(wf[:, :, 2].T.astype(np.float32))  # [CIN, COUT]
    return w12, w3, bf.astype(np.float32)


def _get_nc():
    if "nc" not in _CACHE:
        _CACHE["nc"] = build()
    return _CACHE["nc"]


def make_in_maps(inputs):
    bf = ml_dtypes.bfloat16
    xf = np.asarray(inputs["x"], np.float32).astype(bf)
    x = np.zeros((B, 128, L + 1), dtype=bf)
    x[:, 0:CIN, 0:L] = xf
    x[:, CIN:128, 1 : L + 1] = xf
    folded = {}
    for p in "qkvp":
        key = p if p != "p" else "pe"
        folded[p] = _fold_weights(
            inputs[f"{key}_w"],
            inputs[f"{key}_b"],
            inputs[f"{key}_gamma"],
            inputs[f"{key}_beta"],
            inputs[f"{key}_mean"],
            inputs[f"{key}_var"],
        )
    w12pack = np.concatenate([folded[p][0] for p in "qkvp"], axis=1).astype(bf)
    bcols = np.stack([folded[p][2] for p in "qkvp"], axis=1).astype(np.float32)
    w12e = np.zeros((128, 4 * COUT + 8 + 512 + 128), dtype=bf)
    w12e[:, 4 * COUT + 8 + 512 :] = np.eye(128, dtype=np.float32)
    w12e[:, 0 : 4 * COUT] = w12pack
    w12e[:, 4 * COUT : 4 * COUT + 8] = np.ascontiguousarray(bcols).view(bf)
    w12e[0, 4 * COUT + 8 : 4 * COUT + 8 + 512] = np.tile(folded["v"][2].astype(bf), 4)
    w3e = np.ascontiguousarray(
        np.concatenate([folded[p][1] for p in "qkvp"], axis=1).astype(bf)
    )
    in_maps = []
    for i in range(NCORES):
        m = {
            "x": np.ascontiguousarray(x[i * BP : (i + 1) * BP]),
            "w12e": w12e,
            "w3e": w3e,
        }
        in_maps.append(m)
    return in_maps


def kernel(**inputs):
    nc = _get_nc()
    in_maps = make_in_maps(inputs)
    res = run_bass_kernel_spmd(nc, in_maps, core_ids=list(range(NCORES)))
    out = np.concatenate([res.results[i]["out"] for i in range(NCORES)], axis=0)
    return out.astype(np.float32)


if __name__ == "__main__":
    rng = np.random.default_rng(0)
    ins = {"x": rng.standard_normal((B, CIN, L), dtype=np.float32)}
    for p in ("q", "k", "v", "pe"):
        ins[f"{p}_w"] = (rng.standard_normal((COUT, CIN, KW)) * 0.05).astype(np.float32)
        ins[f"{p}_b"] = (rng.standard_normal(COUT) * 0.05).astype(np.float32)
        ins[f"{p}_gamma"] = rng.uniform(0.5, 1.5, COUT).astype(np.float32)
        ins[f"{p}_beta"] = (rng.standard_normal(COUT) * 0.05).astype(np.float32)
        ins[f"{p}_mean"] = (rng.standard_normal(COUT) * 0.05).astype(np.float32)
        ins[f"{p}_var"] = rng.uniform(0.5, 1.5, COUT).astype(np.float32)
    got = kernel(**ins)
    print("kernel output:", got.shape, got.dtype, np.abs(got).mean())
